# revision 5
# baseline (speedup 1.0000x reference)
"""BitLinear (ternary-quantized linear) Trainium2 kernel.

out = (x @ clip(round(W / scale), -1, 1).T) * scale,  scale = mean(|W|) + 1e-5

Sharding: tensor-parallel over out_features (11008 = 8 * 1376). Every core
receives the full activation x (pre-transposed to [4096, 8192] so the
contraction dim lands on SBUF partitions) plus its own transposed weight
shard [4096, 1376]. Cores quantize their shard on-device and produce a
[8192, 1376] output slice; the host concatenates slices.

Precision: the ternary weights are exact in fp16, so the matmul runs in
fp16 at full TensorEngine rate (1 cycle/row; fp32 would be 4). x is
rounded to fp16 (11-bit mantissa) -> measured ~2e-4 relative error.
Optional KERNEL_MODE=f16hilo splits x = x_hi + x_lo (fp16 each) and does
two accumulating passes for ~5e-7 relative error at 2x PE cost.
"""

import os
import numpy as np

B_, S_, D_, O_ = 4, 2048, 4096, 11008
NCORES = 8
FO = O_ // NCORES            # 1376 out-features per core
TOK = B_ * S_                # 8192 tokens
KT = D_ // 128               # 32 contraction slabs
MB = 256                     # tokens per conversion block
NT = [(0, 512), (512, 512), (1024, 352)]   # n-tile split of FO
EPS = 1e-5

MODE = os.environ.get("KERNEL_MODE", "f16")   # "f16" | "f16hilo"

_cache = {}


def _build_program(n_tokens=TOK, mode=MODE):
    import concourse.bacc as bacc
    import concourse.mybir as mybir
    from concourse import tile

    f32 = mybir.dt.float32
    f16 = mybir.dt.float16
    Alu = mybir.AluOpType
    Act = mybir.ActivationFunctionType

    hilo = mode == "f16hilo"
    nmb = n_tokens // MB

    nc = bacc.Bacc("TRN2", target_bir_lowering=False, debug=False,
                   num_devices=NCORES)

    xt_d = nc.dram_tensor("xt", [D_, n_tokens], f32, kind="ExternalInput")
    wt_d = nc.dram_tensor("wt", [D_, FO], f32, kind="ExternalInput")
    par_d = nc.dram_tensor("params", [128, 4], f32, kind="ExternalInput")
    out_d = nc.dram_tensor("out", [n_tokens, FO], f32, kind="ExternalOutput")

    with tile.TileContext(nc) as tc:
        from contextlib import ExitStack
        with ExitStack() as ctx:
            const = ctx.enter_context(tc.tile_pool(name="const", bufs=1))
            wqpool = ctx.enter_context(tc.tile_pool(name="wq", bufs=1))
            wstage = ctx.enter_context(tc.tile_pool(name="wstage", bufs=2))
            qtmp = ctx.enter_context(tc.tile_pool(name="qtmp", bufs=2))
            xstage = ctx.enter_context(tc.tile_pool(name="xstage", bufs=4))
            xblk = ctx.enter_context(tc.tile_pool(name="xblk", bufs=2))
            outp = ctx.enter_context(tc.tile_pool(name="outp", bufs=2))
            psum = ctx.enter_context(tc.tile_pool(name="psum", bufs=2,
                                                  space="PSUM"))

            pt = const.tile([128, 4], f32)
            nc.sync.dma_start(pt[:], par_d[:])
            scale_ap = pt[:, 0:1]
            b_ap = pt[:, 1:2]
            nb_ap = pt[:, 2:3]

            GK = 4                      # k-slabs per batched x DMA
            NG = KT // GK

            def quantize_slab(k):
                ws = wstage.tile([128, FO], f32, tag="ws", name="ws")
                nc.sync.dma_start(ws[:], wt_d[k * 128:(k + 1) * 128, :])
                qp = qtmp.tile([128, FO], f16, tag="qp", name="qp")
                qn = qtmp.tile([128, FO], f16, tag="qn", name="qn")
                nc.vector.tensor_scalar(out=qp[:], in0=ws[:], scalar1=b_ap,
                                        scalar2=None, op0=Alu.is_ge)
                nc.vector.tensor_scalar(out=qn[:], in0=ws[:], scalar1=nb_ap,
                                        scalar2=None, op0=Alu.is_le)
                wqk = wqpool.tile([128, FO], f16, tag=f"wq{k}", name="wqk")
                nc.vector.tensor_tensor(out=wqk[:], in0=qp[:], in1=qn[:],
                                        op=Alu.subtract)
                return wqk

            def load_x_block(mb):
                xhi = xblk.tile([128, KT, MB], f16, tag="xhi", name="xhi")
                xlo = (xblk.tile([128, KT, MB], f16, tag="xlo", name="xlo")
                       if hilo else None)
                for g in range(NG):
                    xs = xstage.tile([128, GK, MB], f32, tag="xs", name="xs")
                    src = xt_d[g * GK * 128:(g + 1) * GK * 128,
                               mb * MB:(mb + 1) * MB]
                    nc.sync.dma_start(xs[:],
                                      src.rearrange("(g p) m -> p g m", p=128))
                    nc.vector.tensor_copy(xhi[:, g * GK:(g + 1) * GK, :],
                                          xs[:])
                    if hilo:
                        nc.vector.tensor_tensor(
                            out=xlo[:, g * GK:(g + 1) * GK, :], in0=xs[:],
                            in1=xhi[:, g * GK:(g + 1) * GK, :],
                            op=Alu.subtract)
                return xhi, xlo

            # --- prologue: interleave weight quantize with x block 0 so the
            # PE can start as soon as the first slabs land ---
            wq = []
            first_x = None
            for g in range(NG):
                if g == 0:
                    first_x = load_x_block(0)
                for k in range(g * GK, (g + 1) * GK):
                    wq.append(quantize_slab(k))

            # --- main loop: f16 conversion + accumulating matmuls ---
            for mb in range(nmb):
                if mb == 0:
                    xhi, xlo = first_x
                else:
                    xhi, xlo = load_x_block(mb)
                for mt in range(MB // 128):
                    pss = [psum.tile([128, 512], f32, tag=f"ps{j}",
                                     name=f"ps{j}")
                           for j in range(len(NT))]
                    for k in range(KT):
                        lh = xhi[:, k, mt * 128:(mt + 1) * 128]
                        for j, (n0, nw) in enumerate(NT):
                            nc.tensor.matmul(pss[j][:, :nw], lh,
                                             wq[k][:, n0:n0 + nw],
                                             start=(k == 0),
                                             stop=(not hilo and k == KT - 1))
                        if hilo:
                            assert xlo is not None
                            ll = xlo[:, k, mt * 128:(mt + 1) * 128]
                            for j, (n0, nw) in enumerate(NT):
                                nc.tensor.matmul(pss[j][:, :nw], ll,
                                                 wq[k][:, n0:n0 + nw],
                                                 start=False,
                                                 stop=(k == KT - 1))
                    ob = outp.tile([128, FO], f32, tag="ob", name="ob")
                    for j, (n0, nw) in enumerate(NT):
                        nc.scalar.activation(ob[:, n0:n0 + nw],
                                             pss[j][:, :nw], Act.Copy,
                                             scale=scale_ap)
                    row = (mb * (MB // 128) + mt) * 128
                    nc.sync.dma_start(out_d[row:row + 128, :], ob[:])

    nc.compile()
    return nc


def _get_program(n_tokens=TOK, mode=MODE):
    key = (n_tokens, mode)
    if key not in _cache:
        _cache[key] = _build_program(n_tokens, mode)
    return _cache[key]


def _exact_threshold(scale):
    """Smallest fp32 v with fp32(v/scale) > 0.5 (RNE boundary of
    clip(round(w/scale)): w maps to +-1 iff |w/scale| > 0.5 strictly)."""
    scale = np.float32(scale)
    half = np.float32(0.5)
    v = np.float32(half * scale)
    while np.float32(v / scale) > half:
        v = np.nextafter(v, np.float32(0), dtype=np.float32)
    while not (np.float32(v / scale) > half):
        v = np.nextafter(v, np.float32(np.inf), dtype=np.float32)
    return v


LAST_RESULTS = None  # BassKernelResults of the most recent run (for test.py)


def kernel(x, weight):
    from concourse.bass_utils import run_bass_kernel_spmd

    x = np.asarray(x, dtype=np.float32)
    weight = np.asarray(weight, dtype=np.float32)
    n_tokens = x.shape[0] * x.shape[1]

    # scalar scale: replicate reference's fp32 jnp.mean(|W|) + eps.
    try:
        import jax.numpy as jnp
        scale = np.float32(jnp.mean(jnp.abs(jnp.asarray(weight))) + EPS)
    except Exception:
        scale = np.float32(np.float32(np.mean(np.abs(weight),
                                              dtype=np.float64)) + np.float32(EPS))
    bexact = _exact_threshold(scale)

    params = np.zeros((128, 4), np.float32)
    params[:, 0] = scale
    params[:, 1] = bexact
    params[:, 2] = -bexact

    xt = np.ascontiguousarray(x.reshape(n_tokens, D_).T)  # [4096, n_tokens]
    in_maps = []
    for c in range(NCORES):
        wtc = np.ascontiguousarray(weight[c * FO:(c + 1) * FO, :].T)
        in_maps.append({"xt": xt, "wt": wtc, "params": params})

    nc = _get_program(n_tokens, MODE)
    trace = bool(int(os.environ.get("KERNEL_TRACE", "0")))
    res = run_bass_kernel_spmd(nc, in_maps, list(range(NCORES)), trace=trace)
    global LAST_RESULTS
    LAST_RESULTS = res

    out = np.concatenate([res.results[c]["out"] for c in range(NCORES)],
                         axis=1)
    return out.reshape(x.shape[0], x.shape[1], O_)


# revision 7
# speedup vs baseline: 1.0173x; 1.0173x over previous
"""BitLinear (ternary-quantized linear) Trainium2 kernel.

out = (x @ clip(round(W / scale), -1, 1).T) * scale,  scale = mean(|W|) + 1e-5

Sharding: tensor-parallel over out_features (11008 = 8 * 1376). Every core
receives the full activation x (pre-transposed to [4096, 8192] so the
contraction dim lands on SBUF partitions) plus its own transposed weight
shard [4096, 1376]. Cores quantize their shard on-device and produce a
[8192, 1376] output slice; the host concatenates slices.

Precision: the ternary weights are exact in fp16, so the matmul runs in
fp16 at full TensorEngine rate (1 cycle/row; fp32 would be 4). x is
rounded to fp16 (11-bit mantissa) -> measured ~2e-4 relative error.
Optional KERNEL_MODE=f16hilo splits x = x_hi + x_lo (fp16 each) and does
two accumulating passes for ~5e-7 relative error at 2x PE cost.
"""

import os
import numpy as np

B_, S_, D_, O_ = 4, 2048, 4096, 11008
NCORES = 8
FO = O_ // NCORES            # 1376 out-features per core
TOK = B_ * S_                # 8192 tokens
KT = D_ // 128               # 32 contraction slabs
MB = 256                     # tokens per conversion block
NT = [(0, 512), (512, 512), (1024, 352)]   # n-tile split of FO
EPS = 1e-5

MODE = os.environ.get("KERNEL_MODE", "f16")   # "f16" | "f16hilo"

_cache = {}


def _build_program(n_tokens=TOK, mode=MODE):
    import concourse.bacc as bacc
    import concourse.mybir as mybir
    from concourse import tile

    f32 = mybir.dt.float32
    f16 = mybir.dt.float16
    Alu = mybir.AluOpType
    Act = mybir.ActivationFunctionType

    hilo = mode == "f16hilo"
    nmb = n_tokens // MB

    nc = bacc.Bacc("TRN2", target_bir_lowering=False, debug=False,
                   num_devices=NCORES)

    xt_d = nc.dram_tensor("xt", [D_, n_tokens], f32, kind="ExternalInput")
    wt_d = nc.dram_tensor("wt", [D_, FO], f32, kind="ExternalInput")
    par_d = nc.dram_tensor("params", [128, 4], f32, kind="ExternalInput")
    out_d = nc.dram_tensor("out", [n_tokens, FO], f32, kind="ExternalOutput")

    with tile.TileContext(nc) as tc:
        from contextlib import ExitStack
        with ExitStack() as ctx:
            const = ctx.enter_context(tc.tile_pool(name="const", bufs=1))
            wqpool = ctx.enter_context(tc.tile_pool(name="wq", bufs=1))
            wstage = ctx.enter_context(tc.tile_pool(name="wstage", bufs=2))
            qtmp = ctx.enter_context(tc.tile_pool(name="qtmp", bufs=2))
            xstage = ctx.enter_context(tc.tile_pool(name="xstage", bufs=4))
            xblk = ctx.enter_context(tc.tile_pool(name="xblk", bufs=2))
            outp = ctx.enter_context(tc.tile_pool(name="outp", bufs=2))
            psum = ctx.enter_context(tc.tile_pool(name="psum", bufs=2,
                                                  space="PSUM"))

            pt = const.tile([128, 4], f32)
            nc.sync.dma_start(pt[:], par_d[:])
            scale_ap = pt[:, 0:1]
            b_ap = pt[:, 1:2]
            nb_ap = pt[:, 2:3]

            GK = 4                      # k-slabs per batched x DMA
            NG = KT // GK

            def quantize_slab(k):
                ws = wstage.tile([128, FO], f32, tag="ws", name="ws")
                nc.sync.dma_start(ws[:], wt_d[k * 128:(k + 1) * 128, :])
                qp = qtmp.tile([128, FO], f16, tag="qp", name="qp")
                qn = qtmp.tile([128, FO], f16, tag="qn", name="qn")
                nc.vector.tensor_scalar(out=qp[:], in0=ws[:], scalar1=b_ap,
                                        scalar2=None, op0=Alu.is_ge)
                nc.vector.tensor_scalar(out=qn[:], in0=ws[:], scalar1=nb_ap,
                                        scalar2=None, op0=Alu.is_le)
                wqk = wqpool.tile([128, FO], f16, tag=f"wq{k}", name="wqk")
                nc.vector.tensor_tensor(out=wqk[:], in0=qp[:], in1=qn[:],
                                        op=Alu.subtract)
                return wqk

            def load_x_block(mb):
                xhi = xblk.tile([128, KT, MB], f16, tag="xhi", name="xhi")
                xlo = (xblk.tile([128, KT, MB], f16, tag="xlo", name="xlo")
                       if hilo else None)
                for g in range(NG):
                    xs = xstage.tile([128, GK, MB], f32, tag="xs", name="xs")
                    src = xt_d[g * GK * 128:(g + 1) * GK * 128,
                               mb * MB:(mb + 1) * MB]
                    nc.sync.dma_start(xs[:],
                                      src.rearrange("(g p) m -> p g m", p=128))
                    nc.vector.tensor_copy(xhi[:, g * GK:(g + 1) * GK, :],
                                          xs[:])
                    if hilo:
                        nc.vector.tensor_tensor(
                            out=xlo[:, g * GK:(g + 1) * GK, :], in0=xs[:],
                            in1=xhi[:, g * GK:(g + 1) * GK, :],
                            op=Alu.subtract)
                return xhi, xlo

            # --- prologue: weight quantize stream, then x block 0 ---
            wq = [quantize_slab(k) for k in range(KT)]
            first_x = load_x_block(0)

            # --- main loop: f16 conversion + accumulating matmuls ---
            for mb in range(nmb):
                if mb == 0:
                    xhi, xlo = first_x
                else:
                    xhi, xlo = load_x_block(mb)
                for mt in range(MB // 128):
                    pss = [psum.tile([128, 512], f32, tag=f"ps{j}",
                                     name=f"ps{j}")
                           for j in range(len(NT))]
                    for k in range(KT):
                        lh = xhi[:, k, mt * 128:(mt + 1) * 128]
                        for j, (n0, nw) in enumerate(NT):
                            nc.tensor.matmul(pss[j][:, :nw], lh,
                                             wq[k][:, n0:n0 + nw],
                                             start=(k == 0),
                                             stop=(not hilo and k == KT - 1))
                        if hilo:
                            assert xlo is not None
                            ll = xlo[:, k, mt * 128:(mt + 1) * 128]
                            for j, (n0, nw) in enumerate(NT):
                                nc.tensor.matmul(pss[j][:, :nw], ll,
                                                 wq[k][:, n0:n0 + nw],
                                                 start=False,
                                                 stop=(k == KT - 1))
                    ob = outp.tile([128, FO], f32, tag="ob", name="ob")
                    for j, (n0, nw) in enumerate(NT):
                        nc.scalar.activation(ob[:, n0:n0 + nw],
                                             pss[j][:, :nw], Act.Copy,
                                             scale=scale_ap)
                    row = (mb * (MB // 128) + mt) * 128
                    nc.sync.dma_start(out_d[row:row + 128, :], ob[:])

    nc.compile()
    return nc


def _get_program(n_tokens=TOK, mode=MODE):
    key = (n_tokens, mode)
    if key not in _cache:
        _cache[key] = _build_program(n_tokens, mode)
    return _cache[key]


def _exact_threshold(scale):
    """Smallest fp32 v with fp32(v/scale) > 0.5 (RNE boundary of
    clip(round(w/scale)): w maps to +-1 iff |w/scale| > 0.5 strictly)."""
    scale = np.float32(scale)
    half = np.float32(0.5)
    v = np.float32(half * scale)
    while np.float32(v / scale) > half:
        v = np.nextafter(v, np.float32(0), dtype=np.float32)
    while not (np.float32(v / scale) > half):
        v = np.nextafter(v, np.float32(np.inf), dtype=np.float32)
    return v


LAST_RESULTS = None  # BassKernelResults of the most recent run (for test.py)


def kernel(x, weight):
    from concourse.bass_utils import run_bass_kernel_spmd

    x = np.asarray(x, dtype=np.float32)
    weight = np.asarray(weight, dtype=np.float32)
    n_tokens = x.shape[0] * x.shape[1]

    # scalar scale: fp32 mean(|W|) + eps, correctly rounded via an f64
    # accumulator (bit-matches jnp's fp32 mean on this input).
    scale = np.float32(np.float32(np.mean(np.abs(weight), dtype=np.float64))
                       + np.float32(EPS))
    bexact = _exact_threshold(scale)

    params = np.zeros((128, 4), np.float32)
    params[:, 0] = scale
    params[:, 1] = bexact
    params[:, 2] = -bexact

    xt = np.ascontiguousarray(x.reshape(n_tokens, D_).T)  # [4096, n_tokens]
    in_maps = []
    for c in range(NCORES):
        wtc = np.ascontiguousarray(weight[c * FO:(c + 1) * FO, :].T)
        in_maps.append({"xt": xt, "wt": wtc, "params": params})

    nc = _get_program(n_tokens, MODE)
    trace = bool(int(os.environ.get("KERNEL_TRACE", "0")))
    res = run_bass_kernel_spmd(nc, in_maps, list(range(NCORES)), trace=trace)
    global LAST_RESULTS
    LAST_RESULTS = res

    out = np.concatenate([res.results[c]["out"] for c in range(NCORES)],
                         axis=1)
    return out.reshape(x.shape[0], x.shape[1], O_)
